# revision 31
# baseline (speedup 1.0000x reference)
"""1-D Winograd F(2,3) along W for the per-sample adaptive conv.

Per output row pair-of-columns (2tx, 2tx+1):
  d = x_pad[2tx .. 2tx+3]  (V = B^T d computed on device, DVE)
  V0 = d0-d2, V1 = d1+d2, V2 = d2-d1, V3 = d1-d3
  m_pos = sum_{ci,ky} Gw[pos][co,ci,ky] * V_pos[ci, y+ky-1, tx]   (TensorE)
  out[2tx]   = m0+m1+m2      (DVE, from PSUM)
  out[2tx+1] = m1-m2-m3

Host-side prep is layout only (pad, de-interleave into aligned 32-col planes
E0|E1|O0|O1 per row, weight G-transform+transpose); all conv arithmetic of the
reference's contraction happens on device.

MACs: 4 pos x 6 (cic,ky) x 512 out/tile -> 768 matmuls/core vs 1152 direct.
"""

import numpy as np
import ml_dtypes

B, T, CIN, COUT, H, W = 8, 4, 256, 256, 64, 64
KH, KW = 3, 3
NCORES = 8
CH = 2
NTX = W // 2        # 32 winograd tiles per row
HP = H + 2          # 66 padded rows
NPOS = 4
YB_ROWS = 16        # output rows per psum tile -> N = 16*32 = 512
NYB = H // YB_ROWS  # 4

_cache = {}
LAST_EXEC_TIME_NS = None
LAST_PROFILE = None

XROW = 4 * NTX      # 128 cols per padded row: E0|E1|O0|O1


def _build():
    import concourse.mybir as mybir
    import concourse.tile as tile
    from concourse import bacc

    nc = bacc.Bacc(
        "TRN2",
        target_bir_lowering=False,
        debug=False,
        enable_asserts=False,
        num_devices=NCORES,
    )
    x_d = nc.dram_tensor(
        "x", [T, CH, 128, HP * XROW], mybir.dt.bfloat16, kind="ExternalInput"
    ).ap()
    NW = CH * NPOS * CH * KH  # 48 weight tiles
    w_d = nc.dram_tensor(
        "w", [128, NW * 128], mybir.dt.bfloat16, kind="ExternalInput"
    ).ap()
    o_d = nc.dram_tensor(
        "out", [T, CH, 128, H * W], mybir.dt.float32, kind="ExternalOutput"
    ).ap()

    ROW_BLOCKS = [(0, 18), (18, 44), (44, 66)]

    def widx(coc, pos, cic, ky):
        return ((coc * NPOS + pos) * CH + cic) * KH + ky

    with tile.TileContext(nc) as tc:
        with (
            tc.tile_pool(name="persist", bufs=1) as persist,
            tc.tile_pool(name="xv", bufs=2) as xv_pool,
            tc.tile_pool(name="psum", bufs=8, space="PSUM") as psum_pool,
            tc.tile_pool(name="obuf", bufs=4) as out_pool,
        ):
            w_sb = persist.tile([128, NW * 128], mybir.dt.bfloat16, tag="w")

            # x and V tiles, double-buffered across images
            x_sb, v_sb = {}, {}
            for t in range(T):
                for c in range(CH):
                    x_sb[(t, c)] = xv_pool.tile(
                        [128, HP * XROW],
                        mybir.dt.bfloat16,
                        name=f"x{t}{c}",
                        tag=f"x{c}",
                        bufs=2,
                    )
                    for p in range(NPOS):
                        v_sb[(t, c, p)] = xv_pool.tile(
                            [128, HP * NTX],
                            mybir.dt.bfloat16,
                            name=f"v{t}{c}{p}",
                            tag=f"v{c}{p}",
                        )

            # PE warmup on scratch so HAM is at 8/8 when the stream starts
            warm_x = persist.tile([128, 512], mybir.dt.bfloat16, name="warm", tag="warm")
            warm_ps = psum_pool.tile(
                [128, 512], mybir.dt.float32, name="wps", tag="m", bufs=8
            )
            nc.gpsimd.memset(warm_x[:], 0.0)
            for _ in range(16):
                nc.tensor.matmul(
                    warm_ps[:], warm_x[:, :128], warm_x[:], start=True, stop=True
                )

            # DMA order: first-needed slivers first
            def dma_x_block(t, c, blk):
                r0, r1 = ROW_BLOCKS[blk]
                nc.sync.dma_start(
                    x_sb[(t, c)][:, r0 * XROW : r1 * XROW],
                    x_d[t, c, :, r0 * XROW : r1 * XROW],
                )

            nc.sync.dma_start(w_sb[:, : 6 * 128], w_d[:, : 6 * 128])
            dma_x_block(0, 0, 0)
            # chunk 1's first block goes on the (idle) scalar HWDGE ring so it
            # lands in parallel with chunk 0 instead of queued behind it
            r0, r1 = ROW_BLOCKS[0]
            nc.scalar.dma_start(
                x_sb[(0, 1)][:, r0 * XROW : r1 * XROW],
                x_d[0, 1, :, r0 * XROW : r1 * XROW],
            )
            nc.sync.dma_start(w_sb[:, 6 * 128 : 12 * 128], w_d[:, 6 * 128 : 12 * 128])
            nc.sync.dma_start(w_sb[:, 12 * 128 : 24 * 128], w_d[:, 12 * 128 : 24 * 128])
            for blk in (1, 2):
                for c in range(CH):
                    dma_x_block(0, c, blk)
            nc.sync.dma_start(w_sb[:, 24 * 128 :], w_d[:, 24 * 128 :])
            for t in range(1, T):
                for blk in range(3):
                    for c in range(CH):
                        dma_x_block(t, c, blk)

            def v_ops(t, c, blk, poss):
                """emit the input-transform ops for (image, chunk, row-block, positions)"""
                x4 = x_sb[(t, c)][:].rearrange("p (h w) -> p h w", w=XROW)
                vv = [
                    v_sb[(t, c, p)][:].rearrange("p (h w) -> p h w", w=NTX)
                    for p in range(NPOS)
                ]
                r0, r1 = ROW_BLOCKS[blk]
                e0 = x4[:, r0:r1, 0:NTX]
                e1 = x4[:, r0:r1, NTX : 2 * NTX]
                o0 = x4[:, r0:r1, 2 * NTX : 3 * NTX]
                o1 = x4[:, r0:r1, 3 * NTX : 4 * NTX]
                for p in poss:
                    if p == 0:
                        nc.vector.tensor_sub(vv[0][:, r0:r1, :], e0, e1)
                    elif p == 1:
                        nc.vector.tensor_add(vv[1][:, r0:r1, :], o0, e1)
                    elif p == 2:
                        nc.vector.tensor_sub(vv[2][:, r0:r1, :], e1, o0)
                    else:
                        nc.vector.tensor_sub(vv[3][:, r0:r1, :], o0, o1)

            def v_op_chunks(t):
                """input transform for image t as per-(chunk,row-block) thunks,
                interleaved c0/c1 in consumption order"""
                return [
                    (lambda c=c, blk=blk: v_ops(t, c, blk, range(NPOS)))
                    for blk in range(3)
                    for c in range(CH)
                ]

            for p in range(NPOS):
                for c in range(CH):
                    v_ops(0, c, 0, [p])
            for blk in (1, 2):
                for c in range(CH):
                    v_ops(0, c, blk, range(NPOS))
            for t in range(T):
                # next image's transform chunks are interleaved between groups
                # so they don't head-of-line-block this image's output ops on DVE
                pending = v_op_chunks(t + 1) if t + 1 < T else []
                v3 = {
                    (c, p): v_sb[(t, c, p)][:].rearrange("p (h w) -> p h w", w=NTX)
                    for c in range(CH)
                    for p in range(NPOS)
                }
                for coc in range(CH):
                    for yb in range(NYB):
                        y0 = yb * YB_ROWS
                        m = []
                        for p in range(NPOS):
                            mp = psum_pool.tile(
                                [128, YB_ROWS * NTX],
                                mybir.dt.float32,
                                name=f"m{p}",
                                tag="m",
                                bufs=8,
                            )
                            k = 0
                            for cic in range(CH):
                                for ky in range(KH):
                                    idx = widx(coc, p, cic, ky)
                                    nc.tensor.matmul(
                                        mp[:],
                                        w_sb[:, idx * 128 : (idx + 1) * 128],
                                        v3[(cic, p)][
                                            :, y0 + ky : y0 + ky + YB_ROWS, :
                                        ],
                                        start=(k == 0),
                                        stop=(k == CH * KH - 1),
                                    )
                                    k += 1
                            m.append(mp)
                        ob = out_pool.tile([128, 2 * YB_ROWS * NTX], mybir.dt.float32)
                        tmp = out_pool.tile(
                            [128, YB_ROWS * NTX], mybir.dt.float32, name="tmp", tag="tmp"
                        )
                        ev = ob[:, : YB_ROWS * NTX]
                        od = ob[:, YB_ROWS * NTX :]
                        # DVE tensor_tensor allows at most one PSUM operand:
                        # stage m1 into SBUF via the otherwise-idle ScalarE
                        nc.scalar.copy(tmp[:], m[1][:])
                        if not (t == T - 1 and coc == CH - 1 and yb == NYB - 1):
                            nc.vector.tensor_sub(od, tmp[:], m[2][:])
                            nc.vector.tensor_sub(od, od, m[3][:])
                            nc.vector.tensor_add(ev, tmp[:], m[2][:])
                            nc.vector.tensor_add(ev, ev, m[0][:])
                            nc.scalar.dma_start(
                                o_d[t, coc, :, yb * 1024 : (yb + 1) * 1024], ob[:]
                            )
                        else:
                            # final tile: compute+ship the even half before the
                            # last matmuls so only the odd half trails the stream
                            half = YB_ROWS * NTX
                            base = yb * 1024
                            nc.vector.tensor_add(ev, tmp[:], m[2][:])
                            nc.vector.tensor_add(ev, ev, m[0][:])
                            nc.scalar.dma_start(
                                o_d[t, coc, :, base : base + half], ob[:, :half]
                            )
                            nc.vector.tensor_sub(od, tmp[:], m[2][:])
                            nc.vector.tensor_sub(od, od, m[3][:])
                            nc.scalar.dma_start(
                                o_d[t, coc, :, base + half : base + 1024], ob[:, half:]
                            )
                        if pending:
                            pending.pop(0)()

    nc.compile()
    return nc


def _prep_inputs(inputs, ada_weight):
    bf16 = ml_dtypes.bfloat16
    in_maps = []
    for b in range(B):
        xb = inputs[b * T : (b + 1) * T].reshape(T, CH, 128, H, W).astype(bf16)
        xp = np.zeros((T, CH, 128, HP, W + 2), dtype=bf16)
        xp[..., 1 : H + 1, 1 : W + 1] = xb
        # per padded row: E0|E1|O0|O1 planes of 32, all 32-aligned
        xd = np.empty((T, CH, 128, HP, 4, NTX), dtype=bf16)
        xd[..., 0, :] = xp[..., 0:64:2]   # E0 = pad[2tx]
        xd[..., 1, :] = xp[..., 2:66:2]   # E1 = pad[2tx+2]
        xd[..., 2, :] = xp[..., 1:65:2]   # O0 = pad[2tx+1]
        xd[..., 3, :] = xp[..., 3:66:2]  # O1 = pad[2tx+3], cols 3,5,...,65

        # weight transform along kx: g = [w0, (w0+w1+w2)/2, (w0-w1+w2)/2, w2]
        wb = ada_weight[b].astype(np.float32)  # [co, ci, ky, kx]
        g = np.empty((NPOS, COUT, CIN, KH), np.float32)
        g[0] = wb[..., 0]
        g[1] = 0.5 * (wb[..., 0] + wb[..., 1] + wb[..., 2])
        g[2] = 0.5 * (wb[..., 0] - wb[..., 1] + wb[..., 2])
        g[3] = wb[..., 2]
        # -> lhsT tiles [ci, (coc pos cic ky co)]
        gt = g.reshape(NPOS, CH, 128, CH, 128, KH)  # pos coc co cic ci ky
        wprep = gt.transpose(4, 1, 0, 3, 5, 2)  # ci coc pos cic ky co
        wprep = np.ascontiguousarray(wprep.astype(bf16)).reshape(
            128, CH * NPOS * CH * KH * 128
        )
        in_maps.append(
            {"x": xd.reshape(T, CH, 128, HP * XROW), "w": wprep}
        )
    return in_maps


def _unpack_out(res):
    # [T, CH, 128, 4 yb, 2 phase, 16 j, 32 tx] -> [T, C, H, W]
    arr = res.reshape(T, CH, 128, NYB, 2, YB_ROWS, NTX)
    out = np.empty((T, COUT, H, W), np.float32)
    a = arr.transpose(0, 1, 2, 3, 5, 6, 4)  # t ch co yb j tx phase
    out = a.reshape(T, COUT, H, W)
    return out


def _setup_profiling():
    import sys
    import types

    try:
        from antenv.axon_hooks import get_axon_ntff_profile_hook  # noqa: F401

        return
    except ImportError:
        pass
    import antenv
    from trn_agent_boot.trn_boot import _ntff_profile_via_ctypes

    hook = _ntff_profile_via_ctypes("/opt/axon/libaxon_pjrt.so")
    m = types.ModuleType("antenv.axon_hooks")
    m.get_axon_ntff_profile_hook = lambda: hook
    m.set_axon_ntff_profile_hook = lambda h: None
    sys.modules["antenv.axon_hooks"] = m
    antenv.axon_hooks = m

    from concourse import bass_utils

    bass_utils.upload_artifacts = lambda tmpdir: f"file://{tmpdir}"


def kernel(inputs, ada_weight, profile=False, trace_kwargs=None):
    global LAST_EXEC_TIME_NS, LAST_PROFILE
    from concourse.bass_utils import run_bass_kernel_spmd

    if profile:
        _setup_profiling()
    if "nc" not in _cache:
        _cache["nc"] = _build()
    nc = _cache["nc"]

    in_maps = _prep_inputs(np.asarray(inputs), np.asarray(ada_weight))

    kwargs = {}
    if profile:
        kwargs["trace"] = True
        if trace_kwargs:
            kwargs.update(trace_kwargs)
    res = run_bass_kernel_spmd(nc, in_maps, core_ids=list(range(NCORES)), **kwargs)
    if profile:
        LAST_EXEC_TIME_NS = res.exec_time_ns
        LAST_PROFILE = res

    out = np.stack([_unpack_out(res.results[b]["out"]) for b in range(B)])
    return np.ascontiguousarray(out.reshape(B * T, COUT, H, W).astype(np.float32))
